# revision 21
# baseline (speedup 1.0000x reference)
"""Trainium2 Bass kernel for nn_DigitConvolutionalModel (dense_cnn).

Math: the 3x3 valid conv is linear in x, so it folds into fc1:
    conv(x) @ fc1_w.T == x @ (C @ fc1_w.T)  with C [784, 676] the conv matrix.
The whole model is then a 3-layer MLP:
    out = relu(relu(x @ W1 + b1) @ W2 + b2) @ W3 + b3
with W1 = C @ fc1_w.T [784,512], W2 = fc2_w.T [512,512], W3 = out_w.T [512,10].

Sharding: pure data parallelism; batch 32768 -> 8 cores x 4096 rows.

On-chip formulation is fully transposed (features on SBUF partitions, batch on
the free dim), so the big activation matrices never need an on-chip transpose:
the host passes x^T slices, and each layer computes h^T = act(W_l as lhsT, rhs
= h_{l-1}^T) with per-partition bias on the ACT engine.

Per core: batch 4096 in 8 chunks of N=512 (moving free dim / one fp32 PSUM
bank). Layer1 K = 784 = 7 k-tiles of 112 partitions; layers 2/3 K = 512 = 4
k-tiles of 128. Matmul dtype bf16 (fp32 PSUM accumulation): 2.5e-3 L2 rel err.

Perf notes (from NTFF traces):
 - N=512 matmuls issue every ~216ns warm; 384 matmuls ~= 83us is the PE floor.
 - PE pre-warm: dummy matmuls on a zeroed tile while input DMAs stream, so
   HAM is at K=8/8 when the first real matmul issues.
 - Input DMAs split across both HWDGE rings (x chunks on ACT, weights on SP)
   to halve per-DMA ring serialization; ordered by first-use time.
 - Per-chunk output DMAs keep the kernel tail short.
"""

import numpy as np
import ml_dtypes

NCORES = 8
B = 32768
BC = B // NCORES  # rows per core
CH = 512          # batch chunk = matmul moving free dim = one fp32 PSUM bank
NCH = BC // CH
KP1, KT1 = 112, 7  # layer-1 contraction tiling: 784 = 7 * 112
MT1 = 4            # 512 out feats = 4 m-tiles of 128
KT2, MT2 = 4, 4    # layer-2: K=512, M=512
KT3, MO = 4, 10    # layer-3: K=512, M=10
N_WARM = 8         # dummy matmuls to lift HAM to K=8/8 during DMA fill

# "bf16" or "fp32r" (fp32 storage, reduced-precision full-rate matmul)
MM_DTYPE = "bf16"

_cache = {}


def _np_dtype():
    return ml_dtypes.bfloat16 if MM_DTYPE == "bf16" else np.float32


def _build():
    """Trace + compile the Bass program once per process."""
    if "nc" in _cache:
        return _cache["nc"]

    from contextlib import ExitStack

    import concourse.bass as bass
    import concourse.tile as tile
    from concourse import bacc, mybir
    from concourse.bass import ts

    DT = mybir.dt.bfloat16 if MM_DTYPE == "bf16" else mybir.dt.float32r
    F32 = mybir.dt.float32
    Relu = mybir.ActivationFunctionType.Relu
    Ident = mybir.ActivationFunctionType.Identity
    # 2-byte dtype keeps all 8 x^T chunks resident; 4-byte streams through 5
    xt_bufs = NCH if mybir.dt.size(DT) == 2 else 5

    nc = bacc.Bacc(
        "TRN2",
        target_bir_lowering=False,
        debug=False,
        enable_asserts=False,
        num_devices=NCORES,
    )

    # Layouts match SBUF tiles exactly -> per-partition contiguous runs of
    # KT1*CH*2 = 7KB (x chunks), so SDMA engines run at line rate instead of
    # the ~260GB/s that 1KB packets gave.
    xt_d = nc.dram_tensor("xt", [NCH, KP1, KT1 * CH], DT, kind="ExternalInput")
    w1_d = nc.dram_tensor("w1", [KP1, KT1 * 512], DT, kind="ExternalInput")
    w2_d = nc.dram_tensor("w2", [128, KT2 * 512], DT, kind="ExternalInput")
    w3_d = nc.dram_tensor("w3", [128, KT3 * MO], DT, kind="ExternalInput")
    b_d = nc.dram_tensor("b", [128, MT1 + MT2 + 1], F32, kind="ExternalInput")
    out_d = nc.dram_tensor("out", [MO, BC], F32, kind="ExternalOutput")

    with tile.TileContext(nc) as tc, ExitStack() as ctx:
        consts = ctx.enter_context(tc.tile_pool(name="consts", bufs=1))
        xt_pool = ctx.enter_context(tc.tile_pool(name="xt", bufs=xt_bufs))
        h1_pool = ctx.enter_context(tc.tile_pool(name="h1", bufs=8))
        h2_pool = ctx.enter_context(tc.tile_pool(name="h2", bufs=5))
        oc_pool = ctx.enter_context(tc.tile_pool(name="oc", bufs=2))
        ps1 = ctx.enter_context(tc.tile_pool(name="ps1", bufs=5, space="PSUM"))
        ps2 = ctx.enter_context(tc.tile_pool(name="ps2", bufs=2, space="PSUM"))
        ps3 = ctx.enter_context(tc.tile_pool(name="ps3", bufs=1, space="PSUM"))

        # --- PE pre-warm: matmuls on zeroed data while input DMAs stream ---
        warm_sb = consts.tile([128, CH], DT, name="warm_sb")
        nc.gpsimd.memset(warm_sb[:], 0.0)
        warm_ps = ps1.tile([128, CH], F32, name="warm_ps", tag="ps1")
        for _ in range(N_WARM):
            nc.tensor.matmul(
                warm_ps[:], warm_sb[:, :128], warm_sb[:], start=True, stop=True
            )

        # --- input DMAs on three rings, ordered by first PE use ---
        # Layer-1 weights and x chunk 0 are each split k:{0,1} / k:{2..6}
        # so real matmuls can start after ~0.45MB instead of 1.6MB. The
        # first blocks go via SWDGE (gpsimd), whose preamble clears first.
        KA = 2  # k-tiles in the first block
        x0a = consts.tile([KP1, KA * CH], DT, name="x0a")
        nc.scalar.dma_start(x0a[:], xt_d[0][:, : KA * CH])
        w1a = consts.tile([KP1, KA * 512], DT, name="w1a")
        nc.sync.dma_start(w1a[:], w1_d[:, : KA * 512])
        x0b = consts.tile([KP1, (KT1 - KA) * CH], DT, name="x0b")
        nc.scalar.dma_start(x0b[:], xt_d[0][:, KA * CH :])
        w1b = consts.tile([KP1, (KT1 - KA) * 512], DT, name="w1b")
        nc.sync.dma_start(w1b[:], w1_d[:, KA * 512 :])

        xtc = [None] * NCH
        for n in range(1, 6):
            t = xt_pool.tile([KP1, KT1 * CH], DT, name=f"xtc{n}", tag="xtc")
            nc.scalar.dma_start(t[:], xt_d[n])
            xtc[n] = t

        b_sb = consts.tile([128, MT1 + MT2 + 1], F32, name="b_sb")
        nc.sync.dma_start(b_sb[:], b_d[:])
        w2_sb = consts.tile([128, KT2 * 512], DT, name="w2_sb")
        nc.sync.dma_start(w2_sb[:], w2_d[:])
        w3_sb = consts.tile([128, KT3 * MO], DT, name="w3_sb")
        nc.sync.dma_start(w3_sb[:], w3_d[:])
        for n in range(6, NCH):
            t = xt_pool.tile([KP1, KT1 * CH], DT, name=f"xtc{n}", tag="xtc")
            nc.sync.dma_start(t[:], xt_d[n])
            xtc[n] = t

        from concourse.bass import ds

        def w1s(ki, mi):
            if ki < KA:
                return w1a[:, ds(ki * 512 + mi * 128, 128)]
            return w1b[:, ds((ki - KA) * 512 + mi * 128, 128)]

        def x0s(ki):
            if ki < KA:
                return x0a[:, ts(ki, CH)]
            return x0b[:, ts(ki - KA, CH)]

        def layer1(n):
            # one tile per mi so layer-2 matmuls dep only on the ACT they read
            h1t = [
                h1_pool.tile([128, CH], DT, name=f"h1_{n}_{mi}", tag="h1")
                for mi in range(MT1)
            ]
            ps = [
                ps1.tile([128, CH], F32, name=f"ps1_{n}_{mi}", tag="ps1")
                for mi in range(MT1)
            ]
            if n == 0:
                # k-outer: compute starts when the first k-block has landed
                for ki in range(KT1):
                    if ki == KA:
                        # bridge the k{2..6}-block DMA wait with dummy
                        # matmuls so PE stays busy and HAM stays warm
                        for _ in range(14):
                            nc.tensor.matmul(
                                warm_ps[:],
                                warm_sb[:, :128],
                                warm_sb[:],
                                start=True,
                                stop=True,
                            )
                    for mi in range(MT1):
                        nc.tensor.matmul(
                            ps[mi][:],
                            w1s(ki, mi),
                            x0s(ki),
                            start=(ki == 0),
                            stop=(ki == KT1 - 1),
                        )
            else:
                for mi in range(MT1):
                    for ki in range(KT1):
                        nc.tensor.matmul(
                            ps[mi][:],
                            w1s(ki, mi),
                            xtc[n][:, ts(ki, CH)],
                            start=(ki == 0),
                            stop=(ki == KT1 - 1),
                        )
            for mi in range(MT1):
                if mi == MT1 - 1:
                    # last tile on DVE so its consumers unblock ~0.7us sooner
                    nc.vector.tensor_scalar(
                        h1t[mi][:],
                        ps[mi][:],
                        b_sb[:, mi : mi + 1],
                        0.0,
                        op0=mybir.AluOpType.add,
                        op1=mybir.AluOpType.max,
                    )
                else:
                    nc.scalar.activation(
                        h1t[mi][:], ps[mi][:], Relu, bias=b_sb[:, mi : mi + 1]
                    )
            return h1t

        def layer23(n, h1t):
            h2t = [
                h2_pool.tile([128, CH], DT, name=f"h2_{n}_{mi}", tag="h2")
                for mi in range(MT2)
            ]
            for mi in range(MT2):
                ps = ps2.tile([128, CH], F32, name=f"ps2_{n}_{mi}", tag="ps2")
                for ki in range(KT2):
                    nc.tensor.matmul(
                        ps[:],
                        w2_sb[:, ds(ki * 512 + mi * 128, 128)],
                        h1t[ki][:],
                        start=(ki == 0),
                        stop=(ki == KT2 - 1),
                    )
                if mi == MT2 - 1:
                    nc.vector.tensor_scalar(
                        h2t[mi][:],
                        ps[:],
                        b_sb[:, MT1 + mi : MT1 + mi + 1],
                        0.0,
                        op0=mybir.AluOpType.add,
                        op1=mybir.AluOpType.max,
                    )
                else:
                    nc.scalar.activation(
                        h2t[mi][:],
                        ps[:],
                        Relu,
                        bias=b_sb[:, MT1 + mi : MT1 + mi + 1],
                    )
            ps = ps3.tile([MO, CH], F32, name=f"ps3_{n}", tag="ps3")
            for ki in range(KT3):
                nc.tensor.matmul(
                    ps[:],
                    w3_sb[:, ts(ki, MO)],
                    h2t[ki][:],
                    start=(ki == 0),
                    stop=(ki == KT3 - 1),
                )
            oct_ = oc_pool.tile([MO, CH], F32, name=f"oc_{n}", tag="oc")
            nc.scalar.activation(
                oct_[:], ps[:], Ident, bias=b_sb[:MO, MT1 + MT2 : MT1 + MT2 + 1]
            )
            nc.sync.dma_start(out_d[:, ts(n, CH)], oct_[:])

        # Software pipeline: PE never waits on ACT-produced h1 of the same
        # chunk — layer2/3 of chunk n-1 run while layer1 of chunk n fills.
        prev = None
        for n in range(NCH):
            h1t = layer1(n)
            if prev is not None:
                layer23(n - 1, prev)
            prev = h1t
        layer23(NCH - 1, prev)

    nc.compile()
    _cache["nc"] = nc
    return nc


def _prep_inputs(x, conv_w, fc1_w, fc1_b, fc2_w, fc2_b, out_w, out_b):
    dt = _np_dtype()
    f32 = np.float32

    # Conv as a [784, 676] matrix (exact in fp64), folded into fc1.
    C = np.zeros((784, 676), dtype=np.float64)
    oy, ox = np.meshgrid(np.arange(26), np.arange(26), indexing="ij")
    cols = (oy * 26 + ox).ravel()
    for ky in range(3):
        for kx in range(3):
            rows = ((oy + ky) * 28 + (ox + kx)).ravel()
            np.add.at(C, (rows, cols), float(conv_w[ky, kx]))
    W1 = (C @ fc1_w.T.astype(np.float64)).astype(f32)  # [784, 512]

    # [p, t*m] layouts: one contiguous run per SBUF partition
    w1 = np.ascontiguousarray(
        W1.reshape(KT1, KP1, 512).transpose(1, 0, 2)
    ).reshape(KP1, KT1 * 512).astype(dt)
    w2 = np.ascontiguousarray(
        np.ascontiguousarray(fc2_w.T).reshape(KT2, 128, 512).transpose(1, 0, 2)
    ).reshape(128, KT2 * 512).astype(dt)
    # [512,10] -> [4,128,10] -> [128, 4*10] so each partition is one 80B run
    w3 = np.ascontiguousarray(
        np.ascontiguousarray(out_w.T).reshape(KT3, 128, MO).transpose(1, 0, 2)
    ).reshape(128, KT3 * MO).astype(dt)
    b3col = np.zeros((128, 1), dtype=np.float64)
    b3col[:MO, 0] = out_b
    b = np.ascontiguousarray(
        np.concatenate(
            [fc1_b.reshape(MT1, 128).T, fc2_b.reshape(MT2, 128).T, b3col],
            axis=1,
        )
    ).astype(f32)

    in_maps = []
    for c in range(NCORES):
        xc = x[c * BC : (c + 1) * BC].T.astype(dt, order="C")  # [784, BC]
        # -> [chunk, p, t*ch]: chunk DMA is one 7KB run per partition
        xch = np.ascontiguousarray(
            xc.reshape(KT1, KP1, NCH, CH).transpose(2, 1, 0, 3)
        ).reshape(NCH, KP1, KT1 * CH)
        in_maps.append(
            {
                "xt": xch,
                "w1": w1,
                "w2": w2,
                "w3": w3,
                "b": b,
            }
        )
    return in_maps


def kernel(x, conv_w, fc1_w, fc1_b, fc2_w, fc2_b, out_w, out_b, _results=None):
    from concourse.bass_utils import run_bass_kernel_spmd

    # Inputs may arrive as jax arrays; do all host prep in numpy.
    x, conv_w, fc1_w, fc1_b, fc2_w, fc2_b, out_w, out_b = (
        np.asarray(a)
        for a in (x, conv_w, fc1_w, fc1_b, fc2_w, fc2_b, out_w, out_b)
    )
    nc = _build()
    in_maps = _prep_inputs(x, conv_w, fc1_w, fc1_b, fc2_w, fc2_b, out_w, out_b)
    res = run_bass_kernel_spmd(nc, in_maps, core_ids=list(range(NCORES)))
    if _results is not None:
        _results.append(res)
    out = np.empty((B, 10), dtype=np.float32)
    for c in range(NCORES):
        out[c * BC : (c + 1) * BC, :] = res.results[c]["out"].T
    return out


# revision 22
# speedup vs baseline: 1.0487x; 1.0487x over previous
"""Trainium2 Bass kernel for nn_DigitConvolutionalModel (dense_cnn).

Math: the 3x3 valid conv is linear in x, so it folds into fc1:
    conv(x) @ fc1_w.T == x @ (C @ fc1_w.T)  with C [784, 676] the conv matrix.
The whole model is then a 3-layer MLP:
    out = relu(relu(x @ W1 + b1) @ W2 + b2) @ W3 + b3
with W1 = C @ fc1_w.T [784,512], W2 = fc2_w.T [512,512], W3 = out_w.T [512,10].

Sharding: pure data parallelism; batch 32768 -> 8 cores x 4096 rows.

On-chip formulation is fully transposed (features on SBUF partitions, batch on
the free dim), so the big activation matrices never need an on-chip transpose:
the host passes x^T slices, and each layer computes h^T = act(W_l as lhsT, rhs
= h_{l-1}^T) with per-partition bias on the ACT engine.

Per core: batch 4096 in 8 chunks of N=512 (moving free dim / one fp32 PSUM
bank). Layer1 K = 784 = 7 k-tiles of 112 partitions; layers 2/3 K = 512 = 4
k-tiles of 128. Matmul dtype bf16 (fp32 PSUM accumulation): 2.5e-3 L2 rel err.

Perf notes (from NTFF traces):
 - N=512 matmuls issue every ~216ns warm; 384 matmuls ~= 83us is the PE floor.
 - PE pre-warm: dummy matmuls on a zeroed tile while input DMAs stream, so
   HAM is at K=8/8 when the first real matmul issues.
 - Input DMAs split across both HWDGE rings (x chunks on ACT, weights on SP)
   to halve per-DMA ring serialization; ordered by first-use time.
 - Per-chunk output DMAs keep the kernel tail short.
"""

import numpy as np
import ml_dtypes

NCORES = 8
B = 32768
BC = B // NCORES  # rows per core
CH = 512          # batch chunk = matmul moving free dim = one fp32 PSUM bank
NCH = BC // CH
KP1, KT1 = 112, 7  # layer-1 contraction tiling: 784 = 7 * 112
MT1 = 4            # 512 out feats = 4 m-tiles of 128
KT2, MT2 = 4, 4    # layer-2: K=512, M=512
KT3, MO = 4, 10    # layer-3: K=512, M=10
N_WARM = 8         # dummy matmuls to lift HAM to K=8/8 during DMA fill

# "bf16" or "fp32r" (fp32 storage, reduced-precision full-rate matmul)
MM_DTYPE = "bf16"

_cache = {}


def _np_dtype():
    return ml_dtypes.bfloat16 if MM_DTYPE == "bf16" else np.float32


def _build():
    """Trace + compile the Bass program once per process."""
    if "nc" in _cache:
        return _cache["nc"]

    from contextlib import ExitStack

    import concourse.bass as bass
    import concourse.tile as tile
    from concourse import bacc, mybir
    from concourse.bass import ts

    DT = mybir.dt.bfloat16 if MM_DTYPE == "bf16" else mybir.dt.float32r
    F32 = mybir.dt.float32
    Relu = mybir.ActivationFunctionType.Relu
    Ident = mybir.ActivationFunctionType.Identity
    # 2-byte dtype keeps all 8 x^T chunks resident; 4-byte streams through 5
    xt_bufs = NCH if mybir.dt.size(DT) == 2 else 5

    nc = bacc.Bacc(
        "TRN2",
        target_bir_lowering=False,
        debug=False,
        enable_asserts=False,
        num_devices=NCORES,
    )

    # Layouts match SBUF tiles exactly -> per-partition contiguous runs of
    # KT1*CH*2 = 7KB (x chunks), so SDMA engines run at line rate instead of
    # the ~260GB/s that 1KB packets gave.
    xt_d = nc.dram_tensor("xt", [NCH, KP1, KT1 * CH], DT, kind="ExternalInput")
    w1_d = nc.dram_tensor("w1", [KP1, KT1 * 512], DT, kind="ExternalInput")
    w2_d = nc.dram_tensor("w2", [128, KT2 * 512], DT, kind="ExternalInput")
    w3_d = nc.dram_tensor("w3", [128, KT3 * MO], DT, kind="ExternalInput")
    b_d = nc.dram_tensor("b", [128, MT1 + MT2 + 1], F32, kind="ExternalInput")
    out_d = nc.dram_tensor("out", [MO, BC], F32, kind="ExternalOutput")

    with tile.TileContext(nc) as tc, ExitStack() as ctx:
        consts = ctx.enter_context(tc.tile_pool(name="consts", bufs=1))
        xt_pool = ctx.enter_context(tc.tile_pool(name="xt", bufs=xt_bufs))
        h1_pool = ctx.enter_context(tc.tile_pool(name="h1", bufs=8))
        h2_pool = ctx.enter_context(tc.tile_pool(name="h2", bufs=5))
        oc_pool = ctx.enter_context(tc.tile_pool(name="oc", bufs=2))
        ps1 = ctx.enter_context(tc.tile_pool(name="ps1", bufs=5, space="PSUM"))
        ps2 = ctx.enter_context(tc.tile_pool(name="ps2", bufs=2, space="PSUM"))
        ps3 = ctx.enter_context(tc.tile_pool(name="ps3", bufs=1, space="PSUM"))

        # --- PE pre-warm: matmuls on zeroed data while input DMAs stream ---
        warm_sb = consts.tile([128, CH], DT, name="warm_sb")
        nc.gpsimd.memset(warm_sb[:], 0.0)
        warm_ps = ps1.tile([128, CH], F32, name="warm_ps", tag="ps1")
        for _ in range(N_WARM):
            nc.tensor.matmul(
                warm_ps[:], warm_sb[:, :128], warm_sb[:], start=True, stop=True
            )

        # --- input DMAs on both HWDGE rings, ordered by PE deadline ---
        # Chunks 0-2 are split into a k:{0..ka-1} part (one ring) and a
        # k:{ka..6} part (other ring) consumed k-outer, so layer-1 compute
        # tracks DMA arrival instead of waiting for whole chunks. The first
        # ~30us of the kernel is DMA-rate-bound (~265 GB/s aggregate).
        KA = 2   # chunk-0 first-block k-tiles
        KA12 = 4  # chunk-1/2 first-block k-tiles
        xs = {}  # n -> (a_tile, b_tile, ka)

        x0a = consts.tile([KP1, KA * CH], DT, name="x0a")
        nc.scalar.dma_start(x0a[:], xt_d[0][:, : KA * CH])
        w1a = consts.tile([KP1, KA * 512], DT, name="w1a")
        nc.sync.dma_start(w1a[:], w1_d[:, : KA * 512])
        x0b = consts.tile([KP1, (KT1 - KA) * CH], DT, name="x0b")
        nc.scalar.dma_start(x0b[:], xt_d[0][:, KA * CH :])
        w1b = consts.tile([KP1, (KT1 - KA) * 512], DT, name="w1b")
        nc.sync.dma_start(w1b[:], w1_d[:, KA * 512 :])
        xs[0] = (x0a, x0b, KA)

        b_sb = consts.tile([128, MT1 + MT2 + 1], F32, name="b_sb")
        nc.sync.dma_start(b_sb[:], b_d[:])

        x1a = consts.tile([KP1, KA12 * CH], DT, name="x1a")
        nc.scalar.dma_start(x1a[:], xt_d[1][:, : KA12 * CH])
        x1b = consts.tile([KP1, (KT1 - KA12) * CH], DT, name="x1b")
        nc.sync.dma_start(x1b[:], xt_d[1][:, KA12 * CH :])

        w2_sb = consts.tile([128, KT2 * 512], DT, name="w2_sb")
        nc.scalar.dma_start(w2_sb[:], w2_d[:])

        x2a = consts.tile([KP1, KA12 * CH], DT, name="x2a")
        nc.scalar.dma_start(x2a[:], xt_d[2][:, : KA12 * CH])
        x2b = consts.tile([KP1, (KT1 - KA12) * CH], DT, name="x2b")
        nc.sync.dma_start(x2b[:], xt_d[2][:, KA12 * CH :])
        xs[1] = (x1a, x1b, KA12)
        xs[2] = (x2a, x2b, KA12)

        w3_sb = consts.tile([128, KT3 * MO], DT, name="w3_sb")
        nc.sync.dma_start(w3_sb[:], w3_d[:])

        xtc = [None] * NCH
        for n in range(3, NCH):
            t = xt_pool.tile([KP1, KT1 * CH], DT, name=f"xtc{n}", tag="xtc")
            eng = nc.scalar if n < 6 else nc.sync
            eng.dma_start(t[:], xt_d[n])
            xtc[n] = t

        from concourse.bass import ds

        def w1s(ki, mi):
            if ki < KA:
                return w1a[:, ds(ki * 512 + mi * 128, 128)]
            return w1b[:, ds((ki - KA) * 512 + mi * 128, 128)]

        def xslice(n, ki):
            if n in xs:
                a, b_, ka = xs[n]
                if ki < ka:
                    return a[:, ts(ki, CH)]
                return b_[:, ts(ki - ka, CH)]
            return xtc[n][:, ts(ki, CH)]

        def layer1(n):
            # one tile per mi so layer-2 matmuls dep only on the ACT they read
            h1t = [
                h1_pool.tile([128, CH], DT, name=f"h1_{n}_{mi}", tag="h1")
                for mi in range(MT1)
            ]
            ps = [
                ps1.tile([128, CH], F32, name=f"ps1_{n}_{mi}", tag="ps1")
                for mi in range(MT1)
            ]
            if n in xs:
                # k-outer: compute starts as soon as each k-block lands
                for ki in range(KT1):
                    if n == 0 and ki == KA:
                        # bridge the k{2..6}-block DMA wait with dummy
                        # matmuls so PE stays busy and HAM stays warm
                        for _ in range(14):
                            nc.tensor.matmul(
                                warm_ps[:],
                                warm_sb[:, :128],
                                warm_sb[:],
                                start=True,
                                stop=True,
                            )
                    for mi in range(MT1):
                        nc.tensor.matmul(
                            ps[mi][:],
                            w1s(ki, mi),
                            xslice(n, ki),
                            start=(ki == 0),
                            stop=(ki == KT1 - 1),
                        )
            else:
                for mi in range(MT1):
                    for ki in range(KT1):
                        nc.tensor.matmul(
                            ps[mi][:],
                            w1s(ki, mi),
                            xslice(n, ki),
                            start=(ki == 0),
                            stop=(ki == KT1 - 1),
                        )
            for mi in range(MT1):
                nc.scalar.activation(
                    h1t[mi][:], ps[mi][:], Relu, bias=b_sb[:, mi : mi + 1]
                )
            return h1t

        def layer23(n, h1t):
            h2t = [
                h2_pool.tile([128, CH], DT, name=f"h2_{n}_{mi}", tag="h2")
                for mi in range(MT2)
            ]
            for mi in range(MT2):
                ps = ps2.tile([128, CH], F32, name=f"ps2_{n}_{mi}", tag="ps2")
                for ki in range(KT2):
                    nc.tensor.matmul(
                        ps[:],
                        w2_sb[:, ds(ki * 512 + mi * 128, 128)],
                        h1t[ki][:],
                        start=(ki == 0),
                        stop=(ki == KT2 - 1),
                    )
                nc.scalar.activation(
                    h2t[mi][:],
                    ps[:],
                    Relu,
                    bias=b_sb[:, MT1 + mi : MT1 + mi + 1],
                )
            ps = ps3.tile([MO, CH], F32, name=f"ps3_{n}", tag="ps3")
            for ki in range(KT3):
                nc.tensor.matmul(
                    ps[:],
                    w3_sb[:, ts(ki, MO)],
                    h2t[ki][:],
                    start=(ki == 0),
                    stop=(ki == KT3 - 1),
                )
            oct_ = oc_pool.tile([MO, CH], F32, name=f"oc_{n}", tag="oc")
            nc.scalar.activation(
                oct_[:], ps[:], Ident, bias=b_sb[:MO, MT1 + MT2 : MT1 + MT2 + 1]
            )
            nc.sync.dma_start(out_d[:, ts(n, CH)], oct_[:])

        # Software pipeline: PE never waits on ACT-produced h1 of the same
        # chunk — layer2/3 of chunk n-1 run while layer1 of chunk n fills.
        prev = None
        for n in range(NCH):
            h1t = layer1(n)
            if prev is not None:
                layer23(n - 1, prev)
            prev = h1t
        layer23(NCH - 1, prev)

    nc.compile()
    _cache["nc"] = nc
    return nc


def _prep_inputs(x, conv_w, fc1_w, fc1_b, fc2_w, fc2_b, out_w, out_b):
    dt = _np_dtype()
    f32 = np.float32

    # Conv as a [784, 676] matrix (exact in fp64), folded into fc1.
    C = np.zeros((784, 676), dtype=np.float64)
    oy, ox = np.meshgrid(np.arange(26), np.arange(26), indexing="ij")
    cols = (oy * 26 + ox).ravel()
    for ky in range(3):
        for kx in range(3):
            rows = ((oy + ky) * 28 + (ox + kx)).ravel()
            np.add.at(C, (rows, cols), float(conv_w[ky, kx]))
    W1 = (C @ fc1_w.T.astype(np.float64)).astype(f32)  # [784, 512]

    # [p, t*m] layouts: one contiguous run per SBUF partition
    w1 = np.ascontiguousarray(
        W1.reshape(KT1, KP1, 512).transpose(1, 0, 2)
    ).reshape(KP1, KT1 * 512).astype(dt)
    w2 = np.ascontiguousarray(
        np.ascontiguousarray(fc2_w.T).reshape(KT2, 128, 512).transpose(1, 0, 2)
    ).reshape(128, KT2 * 512).astype(dt)
    # [512,10] -> [4,128,10] -> [128, 4*10] so each partition is one 80B run
    w3 = np.ascontiguousarray(
        np.ascontiguousarray(out_w.T).reshape(KT3, 128, MO).transpose(1, 0, 2)
    ).reshape(128, KT3 * MO).astype(dt)
    b3col = np.zeros((128, 1), dtype=np.float64)
    b3col[:MO, 0] = out_b
    b = np.ascontiguousarray(
        np.concatenate(
            [fc1_b.reshape(MT1, 128).T, fc2_b.reshape(MT2, 128).T, b3col],
            axis=1,
        )
    ).astype(f32)

    in_maps = []
    for c in range(NCORES):
        xc = x[c * BC : (c + 1) * BC].T.astype(dt, order="C")  # [784, BC]
        # -> [chunk, p, t*ch]: chunk DMA is one 7KB run per partition
        xch = np.ascontiguousarray(
            xc.reshape(KT1, KP1, NCH, CH).transpose(2, 1, 0, 3)
        ).reshape(NCH, KP1, KT1 * CH)
        in_maps.append(
            {
                "xt": xch,
                "w1": w1,
                "w2": w2,
                "w3": w3,
                "b": b,
            }
        )
    return in_maps


def kernel(x, conv_w, fc1_w, fc1_b, fc2_w, fc2_b, out_w, out_b, _results=None):
    from concourse.bass_utils import run_bass_kernel_spmd

    # Inputs may arrive as jax arrays; do all host prep in numpy.
    x, conv_w, fc1_w, fc1_b, fc2_w, fc2_b, out_w, out_b = (
        np.asarray(a)
        for a in (x, conv_w, fc1_w, fc1_b, fc2_w, fc2_b, out_w, out_b)
    )
    nc = _build()
    in_maps = _prep_inputs(x, conv_w, fc1_w, fc1_b, fc2_w, fc2_b, out_w, out_b)
    res = run_bass_kernel_spmd(nc, in_maps, core_ids=list(range(NCORES)))
    if _results is not None:
        _results.append(res)
    out = np.empty((B, 10), dtype=np.float32)
    for c in range(NCORES):
        out[c * BC : (c + 1) * BC, :] = res.results[c]["out"].T
    return out
